# revision 5
# baseline (speedup 1.0000x reference)
"""Trainium2 Bass kernel for ContrastiveSemLoss.

Math (reference):
    fea = feature[:100]                        # [n=100, d=512]
    v[s, i] = sum_d attribute_vec[d, s] * fea[i, d]
    ex[s, i] = v[s, i] / (TAL * |a_s| * |f_i|)
    loss = mean_i( logsumexp_s ex[:, i] - ex[label_i, i] )

Sharding: attribute_vec split column-wise (class dim s_n=150000) across
8 cores, 18750 classes each.  Each core streams its [512, 18750] shard
from HBM once and computes, per class chunk of 512:
    - logits q[i, s] = sum_d fwT[d, i] * a[d, s]  (PE, fp32r)
      where fwT = fea.T / (TAL * |f_i|)  (folded host-side)
    - class sumsq ss[s]  = ones.T @ (a*a)         (PE, fp32r)
    - inv_norm = 1/sqrt(ss)                       (ACT sqrt + DVE recip)
    - broadcast inv_norm over partitions          (GPSIMD)
    - ex = q * inv_norm ; partial Z = sum_s exp(ex)  (DVE mul, ACT exp+accum)
The positive logit ex[label_i, i] is computed on-device from the 100
gathered attribute columns (gathered host-side while sharding).
Host combines: loss = mean(log(sum_c Z_c) - pos).
"""

import sys

if "/opt/trn_rl_repo" not in sys.path:
    sys.path.insert(0, "/opt/trn_rl_repo")

import numpy as np

import concourse.bacc as bacc
import concourse.tile as tile
from concourse import mybir
from concourse.bass_utils import run_bass_kernel_spmd

TAL = 0.07
N_USE = 100
D = 512
S_N = 150000
NCORES = 8
S_SHARD = S_N // NCORES          # 18750
F = 512                          # class chunk (psum bank free size)
KC = D // 128                    # 4 d-chunks of 128
NCH = (S_SHARD + F - 1) // F     # 37 chunks (36 full + 318)

F32 = mybir.dt.float32
F32R = mybir.dt.float32r
BF16 = mybir.dt.bfloat16
AF = mybir.ActivationFunctionType


def _build():
    nc = bacc.Bacc("TRN2", target_bir_lowering=False, debug=False,
                   num_devices=NCORES)
    ash = nc.dram_tensor("ash", [D, S_SHARD], F32R, kind="ExternalInput").ap()
    fw = nc.dram_tensor("fw", [D, N_USE], F32R, kind="ExternalInput").ap()
    aselt = nc.dram_tensor("aselt", [N_USE, D], F32, kind="ExternalInput").ap()
    feasc = nc.dram_tensor("feasc", [N_USE, D], F32, kind="ExternalInput").ap()
    zout = nc.dram_tensor("zout", [N_USE], F32, kind="ExternalOutput").ap()
    posout = nc.dram_tensor("posout", [N_USE], F32, kind="ExternalOutput").ap()

    ash_r = ash.rearrange("(c p) s -> p c s", p=128)   # [128, 4, 18750]
    fw_r = fw.rearrange("(c p) n -> p c n", p=128)     # [128, 4, 100]

    with tile.TileContext(nc) as tc:
        with (
            tc.tile_pool(name="const", bufs=1) as const,
            tc.tile_pool(name="loads", bufs=3) as loads,
            tc.tile_pool(name="sqp", bufs=3) as sqp,
            tc.tile_pool(name="rows", bufs=3) as rows,
            tc.tile_pool(name="bcp", bufs=3) as bcp,
            tc.tile_pool(name="exi", bufs=3) as exi,
            tc.tile_pool(name="exo", bufs=2) as exo,
            tc.tile_pool(name="psv", bufs=2, space="PSUM") as psv,
            tc.tile_pool(name="pss", bufs=2, space="PSUM") as pssp,
        ):
            # ---- constants ----
            fw_sb = const.tile([128, KC, N_USE], F32R)
            nc.sync.dma_start(out=fw_sb, in_=fw_r)
            ones = const.tile([128, 1], BF16)
            nc.vector.memset(ones, 1.0)
            zacc = const.tile([N_USE, NCH], F32)

            # ---- positive-logit epilogue (tiny, independent) ----
            asel_sb = const.tile([N_USE, D], F32)
            feasc_sb = const.tile([N_USE, D], F32)
            nc.sync.dma_start(out=asel_sb, in_=aselt)
            nc.sync.dma_start(out=feasc_sb, in_=feasc)
            scr0 = const.tile([N_USE, D], F32)
            scr1 = const.tile([N_USE, D], F32)
            qpos = const.tile([N_USE, 1], F32)
            sspos = const.tile([N_USE, 1], F32)
            nc.vector.tensor_mul(scr0, asel_sb, feasc_sb)
            nc.vector.reduce_sum(out=qpos, in_=scr0,
                                 axis=mybir.AxisListType.X,
                                 op=mybir.AluOpType.add)
            nc.vector.tensor_mul(scr1, asel_sb, asel_sb)
            nc.vector.reduce_sum(out=sspos, in_=scr1,
                                 axis=mybir.AxisListType.X,
                                 op=mybir.AluOpType.add)
            apos = const.tile([N_USE, 1], F32)
            nc.scalar.activation(out=apos, in_=sspos, func=AF.Sqrt)
            iapos = const.tile([N_USE, 1], F32)
            nc.vector.reciprocal(out=iapos, in_=apos)
            pos_t = const.tile([N_USE, 1], F32)
            nc.vector.tensor_mul(pos_t, qpos, iapos)
            nc.sync.dma_start(out=posout.unsqueeze(1), in_=pos_t)

            # ---- main loop over class chunks ----
            for j in range(NCH):
                s0 = j * F
                fj = min(F, S_SHARD - s0)
                a_t = loads.tile([128, KC, F], F32R, tag="a")
                nc.sync.dma_start(out=a_t[:, :, :fj],
                                  in_=ash_r[:, :, s0:s0 + fj])
                sq = sqp.tile([128, KC, F], BF16, tag="sq")
                for c in range(KC):
                    if c % 2 == 0:
                        nc.vector.tensor_mul(sq[:, c, :fj],
                                             a_t[:, c, :fj].bitcast(F32),
                                             a_t[:, c, :fj].bitcast(F32))
                    else:
                        nc.scalar.activation(out=sq[:, c, :fj],
                                             in_=a_t[:, c, :fj].bitcast(F32),
                                             func=AF.Square)
                pv = psv.tile([N_USE, F], F32, tag="pv")
                for c in range(KC):
                    nc.tensor.matmul(pv[:, :fj],
                                     fw_sb[:, c, :],
                                     a_t[:, c, :fj],
                                     start=(c == 0), stop=(c == KC - 1))
                ps = pssp.tile([1, F], F32, tag="ps")
                for c in range(KC):
                    nc.tensor.matmul(ps[:, :fj],
                                     ones,
                                     sq[:, c, :fj],
                                     start=(c == 0), stop=(c == KC - 1))
                arow = rows.tile([1, F], F32, tag="arow")
                nc.scalar.activation(out=arow[:, :fj], in_=ps[:, :fj],
                                     func=AF.Sqrt)
                inva = rows.tile([1, F], F32, tag="inva")
                nc.vector.reciprocal(out=inva[:, :fj], in_=arow[:, :fj])
                bca = bcp.tile([N_USE, F], F32, tag="bca")
                nc.gpsimd.partition_broadcast(bca[:, :fj], inva[:, :fj])
                ext = exi.tile([N_USE, F], F32, tag="ext")
                nc.vector.tensor_mul(ext[:, :fj], pv[:, :fj], bca[:, :fj])
                eo = exo.tile([N_USE, F], F32, tag="eo")
                nc.scalar.activation(out=eo[:, :fj], in_=ext[:, :fj],
                                     func=AF.Exp,
                                     accum_out=zacc[:, j:j + 1])

            zf = const.tile([N_USE, 1], F32)
            nc.vector.reduce_sum(out=zf, in_=zacc,
                                 axis=mybir.AxisListType.X,
                                 op=mybir.AluOpType.add)
            nc.sync.dma_start(out=zout.unsqueeze(1), in_=zf)

    nc.compile()
    return nc


_NC_CACHE = None


def _get_nc():
    global _NC_CACHE
    if _NC_CACHE is None:
        _NC_CACHE = _build()
    return _NC_CACHE


def _prep_inputs(attribute_vec, feature, label):
    a = np.asarray(attribute_vec, dtype=np.float32)
    fea = np.asarray(feature, dtype=np.float32)[:N_USE]
    lab = np.asarray(label)[:N_USE].astype(np.int64)

    fea_norm = np.linalg.norm(fea, axis=1).astype(np.float32)      # [100]
    scale = (1.0 / (TAL * fea_norm)).astype(np.float32)            # [100]
    fw = np.ascontiguousarray(fea.T * scale[None, :], dtype=np.float32)
    asel = a[:, lab]                                               # [512,100]
    aselt = np.ascontiguousarray(asel.T, dtype=np.float32)         # [100,512]
    feasc = np.ascontiguousarray(fea * scale[:, None], dtype=np.float32)

    in_maps = []
    for c in range(NCORES):
        sh = np.ascontiguousarray(a[:, c * S_SHARD:(c + 1) * S_SHARD])
        in_maps.append({"ash": sh, "fw": fw, "aselt": aselt, "feasc": feasc})
    return in_maps


def _combine(results):
    z = np.stack([results[c]["zout"] for c in range(NCORES)], axis=0)  # [8,100]
    pos = results[0]["posout"]                                         # [100]
    zsum = z.astype(np.float64).sum(axis=0)
    lse = np.log(zsum)
    loss = float(np.mean(lse - pos.astype(np.float64)))
    return np.float32(loss)


def kernel(**inputs):
    in_maps = _prep_inputs(inputs["attribute_vec"], inputs["feature"],
                           inputs["label"])
    nc = _get_nc()
    res = run_bass_kernel_spmd(nc, in_maps, list(range(NCORES)))
    return _combine(res.results)


if __name__ == "__main__":
    rng = np.random.default_rng(0)
    a = rng.standard_normal((D, S_N), dtype=np.float32)
    f = rng.standard_normal((4096, D), dtype=np.float32)
    l = rng.integers(0, S_N, size=(4096,)).astype(np.int64)
    out = kernel(attribute_vec=a, feature=f, label=l)
    print("loss:", out)


# revision 7
# speedup vs baseline: 1.1530x; 1.1530x over previous
"""Trainium2 Bass kernel for ContrastiveSemLoss.

Math (reference):
    fea = feature[:100]                        # [n=100, d=512]
    v[s, i] = sum_d attribute_vec[d, s] * fea[i, d]
    ex[s, i] = v[s, i] / (TAL * |a_s| * |f_i|)
    loss = mean_i( logsumexp_s ex[:, i] - ex[label_i, i] )

Sharding: attribute_vec split column-wise (class dim s_n=150000) across
8 cores, 18750 classes each.  Each core streams its [512, 18750] shard
from HBM once and computes, per class chunk of 512:
    - logits q[i, s] = sum_d fwT[d, i] * a[d, s]  (PE, fp32r)
      where fwT = fea.T / (TAL * |f_i|)  (folded host-side)
    - class sumsq ss[s]  = ones.T @ (a*a)         (PE, fp32r)
    - inv_norm = 1/sqrt(ss)                       (ACT sqrt + DVE recip)
    - broadcast inv_norm over partitions          (GPSIMD)
    - ex = q * inv_norm ; partial Z = sum_s exp(ex)  (DVE mul, ACT exp+accum)
The positive logit ex[label_i, i] is computed on-device from the 100
gathered attribute columns (gathered host-side while sharding).
Host combines: loss = mean(log(sum_c Z_c) - pos).
"""

import sys

if "/opt/trn_rl_repo" not in sys.path:
    sys.path.insert(0, "/opt/trn_rl_repo")

import numpy as np

import concourse.bacc as bacc
import concourse.tile as tile
from concourse import mybir
from concourse.bass_utils import run_bass_kernel_spmd

TAL = 0.07
N_USE = 100
D = 512
S_N = 150000
NCORES = 8
S_SHARD = S_N // NCORES          # 18750
F = 512                          # class chunk (psum bank free size)
KC = D // 128                    # 4 d-chunks of 128
NCH = (S_SHARD + F - 1) // F     # 37 chunks (36 full + 318)

F32 = mybir.dt.float32
F32R = mybir.dt.float32r
BF16 = mybir.dt.bfloat16
AF = mybir.ActivationFunctionType


def _build():
    nc = bacc.Bacc("TRN2", target_bir_lowering=False, debug=False,
                   num_devices=NCORES)
    ash = nc.dram_tensor("ash", [D, S_SHARD], F32R, kind="ExternalInput").ap()
    fw = nc.dram_tensor("fw", [D, N_USE], F32R, kind="ExternalInput").ap()
    aselt = nc.dram_tensor("aselt", [N_USE, D], F32, kind="ExternalInput").ap()
    feasc = nc.dram_tensor("feasc", [N_USE, D], F32, kind="ExternalInput").ap()
    zout = nc.dram_tensor("zout", [N_USE], F32, kind="ExternalOutput").ap()
    posout = nc.dram_tensor("posout", [N_USE], F32, kind="ExternalOutput").ap()

    ash_r = ash.rearrange("(c p) s -> p c s", p=128)   # [128, 4, 18750]
    fw_r = fw.rearrange("(c p) n -> p c n", p=128)     # [128, 4, 100]

    with tile.TileContext(nc) as tc:
        with (
            tc.tile_pool(name="const", bufs=1) as const,
            tc.tile_pool(name="loads", bufs=3) as loads,
            tc.tile_pool(name="sqp", bufs=3) as sqp,
            tc.tile_pool(name="rows", bufs=3) as rows,
            tc.tile_pool(name="bcp", bufs=3) as bcp,
            tc.tile_pool(name="exi", bufs=3) as exi,
            tc.tile_pool(name="exo", bufs=2) as exo,
            tc.tile_pool(name="psv", bufs=2, space="PSUM") as psv,
            tc.tile_pool(name="pss", bufs=2, space="PSUM") as pssp,
        ):
            # ---- constants ----
            fw_sb = const.tile([128, KC, N_USE], F32R)
            nc.sync.dma_start(out=fw_sb, in_=fw_r)
            ones = const.tile([128, 1], BF16)
            nc.vector.memset(ones, 1.0)
            zacc = const.tile([N_USE, NCH], F32)

            # ---- positive-logit epilogue (tiny, independent) ----
            asel_sb = const.tile([N_USE, D], F32)
            feasc_sb = const.tile([N_USE, D], F32)
            nc.sync.dma_start(out=asel_sb, in_=aselt)
            nc.sync.dma_start(out=feasc_sb, in_=feasc)
            scr0 = const.tile([N_USE, D], F32)
            scr1 = const.tile([N_USE, D], F32)
            qpos = const.tile([N_USE, 1], F32)
            sspos = const.tile([N_USE, 1], F32)
            nc.vector.tensor_mul(scr0, asel_sb, feasc_sb)
            nc.vector.reduce_sum(out=qpos, in_=scr0,
                                 axis=mybir.AxisListType.X,
                                 op=mybir.AluOpType.add)
            nc.vector.tensor_mul(scr1, asel_sb, asel_sb)
            nc.vector.reduce_sum(out=sspos, in_=scr1,
                                 axis=mybir.AxisListType.X,
                                 op=mybir.AluOpType.add)
            apos = const.tile([N_USE, 1], F32)
            nc.scalar.activation(out=apos, in_=sspos, func=AF.Ln)
            iapos = const.tile([N_USE, 1], F32)
            nc.scalar.activation(out=iapos, in_=apos, func=AF.Exp, scale=-0.5)
            pos_t = const.tile([N_USE, 1], F32)
            nc.vector.tensor_mul(pos_t, qpos, iapos)
            nc.sync.dma_start(out=posout.unsqueeze(1), in_=pos_t)

            # ---- main loop over class chunks ----
            for j in range(NCH):
                s0 = j * F
                fj = min(F, S_SHARD - s0)
                a_t = loads.tile([128, KC, F], F32R, tag="a")
                nc.sync.dma_start(out=a_t[:, :, :fj],
                                  in_=ash_r[:, :, s0:s0 + fj])
                sq = sqp.tile([128, KC, F], BF16, tag="sq")
                for c in range(KC):
                    if c % 2 == 0:
                        nc.vector.tensor_mul(sq[:, c, :fj],
                                             a_t[:, c, :fj].bitcast(F32),
                                             a_t[:, c, :fj].bitcast(F32))
                    else:
                        nc.scalar.activation(out=sq[:, c, :fj],
                                             in_=a_t[:, c, :fj].bitcast(F32),
                                             func=AF.Square)
                pv = psv.tile([N_USE, F], F32, tag="pv")
                for c in range(KC):
                    nc.tensor.matmul(pv[:, :fj],
                                     fw_sb[:, c, :],
                                     a_t[:, c, :fj],
                                     start=(c == 0), stop=(c == KC - 1))
                ps = pssp.tile([1, F], F32, tag="ps")
                for c in range(KC):
                    nc.tensor.matmul(ps[:, :fj],
                                     ones,
                                     sq[:, c, :fj],
                                     start=(c == 0), stop=(c == KC - 1))
                arow = rows.tile([1, F], F32, tag="arow")
                nc.scalar.activation(out=arow[:, :fj], in_=ps[:, :fj],
                                     func=AF.Ln)
                inva = rows.tile([1, F], F32, tag="inva")
                nc.scalar.activation(out=inva[:, :fj], in_=arow[:, :fj],
                                     func=AF.Exp, scale=-0.5)
                bca = bcp.tile([N_USE, F], F32, tag="bca")
                nc.gpsimd.partition_broadcast(bca[:, :fj], inva[:, :fj])
                ext = exi.tile([N_USE, F], F32, tag="ext")
                nc.vector.tensor_mul(ext[:, :fj], pv[:, :fj], bca[:, :fj])
                eo = exo.tile([N_USE, F], F32, tag="eo")
                nc.scalar.activation(out=eo[:, :fj], in_=ext[:, :fj],
                                     func=AF.Exp,
                                     accum_out=zacc[:, j:j + 1])

            zf = const.tile([N_USE, 1], F32)
            nc.vector.reduce_sum(out=zf, in_=zacc,
                                 axis=mybir.AxisListType.X,
                                 op=mybir.AluOpType.add)
            nc.sync.dma_start(out=zout.unsqueeze(1), in_=zf)

    nc.compile()
    return nc


_NC_CACHE = None


def _get_nc():
    global _NC_CACHE
    if _NC_CACHE is None:
        _NC_CACHE = _build()
    return _NC_CACHE


def _prep_inputs(attribute_vec, feature, label):
    a = np.asarray(attribute_vec, dtype=np.float32)
    fea = np.asarray(feature, dtype=np.float32)[:N_USE]
    lab = np.asarray(label)[:N_USE].astype(np.int64)

    fea_norm = np.linalg.norm(fea, axis=1).astype(np.float32)      # [100]
    scale = (1.0 / (TAL * fea_norm)).astype(np.float32)            # [100]
    fw = np.ascontiguousarray(fea.T * scale[None, :], dtype=np.float32)
    asel = a[:, lab]                                               # [512,100]
    aselt = np.ascontiguousarray(asel.T, dtype=np.float32)         # [100,512]
    feasc = np.ascontiguousarray(fea * scale[:, None], dtype=np.float32)

    in_maps = []
    for c in range(NCORES):
        sh = np.ascontiguousarray(a[:, c * S_SHARD:(c + 1) * S_SHARD])
        in_maps.append({"ash": sh, "fw": fw, "aselt": aselt, "feasc": feasc})
    return in_maps


def _combine(results):
    z = np.stack([results[c]["zout"] for c in range(NCORES)], axis=0)  # [8,100]
    pos = results[0]["posout"]                                         # [100]
    zsum = z.astype(np.float64).sum(axis=0)
    lse = np.log(zsum)
    loss = float(np.mean(lse - pos.astype(np.float64)))
    return np.float32(loss)


def kernel(**inputs):
    in_maps = _prep_inputs(inputs["attribute_vec"], inputs["feature"],
                           inputs["label"])
    nc = _get_nc()
    res = run_bass_kernel_spmd(nc, in_maps, list(range(NCORES)))
    return _combine(res.results)


if __name__ == "__main__":
    rng = np.random.default_rng(0)
    a = rng.standard_normal((D, S_N), dtype=np.float32)
    f = rng.standard_normal((4096, D), dtype=np.float32)
    l = rng.integers(0, S_N, size=(4096,)).astype(np.int64)
    out = kernel(attribute_vec=a, feature=f, label=l)
    print("loss:", out)
